# revision 2
# baseline (speedup 1.0000x reference)
"""BallLoss Trainium2 kernel v6 (8-core data-parallel SPMD).

loss = sum_{i,j} relu(d_i - d_ij),  d_ij = ||e_i - c_j||, d_i = d_{i,label_i}

Per-core formulation (rows sharded along N across 8 cores, centers
replicated), using the identity
    sum_j relu(d_i - d_ij) = C*d_i - sum_j min(d_ij, d_i).

Per [128, 2048] row-tile:
  - PE:   full d2[i,j] = e2_i + c2_j - 2*e_i.c_j via one augmented bf16
          matmul: lhsT = [e; e2; 1]^T (stationary, K=66),
          rhs = [-2c; 1; c2]^T. Both norm terms ride the contraction,
          so no ACT bias is needed.
  - ACT:  dist = sqrt(psum), PSUM -> SBUF bf16, one op per tile
          (~1.85us). ACT and PE are the pacing engines.
  - DVE (most tiles): u = min(dist_left, d_i) at 4x over the left half,
          then scalar_tensor_tensor computes min(dist_right, d_i) + u
          with accum_out -> macc[:, t] = sum_j min(d_ij, d_i). The
          accum variant runs 1x but only over half the tile.
  - ACT-assist tiles (balance): DVE min at 4x over the full tile,
          ACT Identity + accum_out does the sum.

d_i arrives precomputed from the host (exact fp32 ||e_i - c_lab_i||,
one numpy gather+norm inside kernel()): the on-device alternatives all
lose — per-tile indirect-DMA gathers are throttled by DMA-ring credits
to ~2.2-2.8us/tile (slower than the tile pace, stalling the in-order
DVE/ACT queues through the dist pool), multi-offset batched indirect
DMA corrupts data on HW, and the dma_gather ext-isa wedges the device
in this runtime. No gpsimd compute at all (gpsimd elementwise stalls
2-port DVE ops on the shared SBUF port).

chat build on device: -2c from a bf16 c^T load (exact scaling), then
csqb = chat*chat = 4c^2 and a 0.25-ones colsum matmul gives c2; the
ones row comes from a host constant.

Final: rowloss = C*dall - macc (small stt), DVE free reduce + PE
ones-matmul partition reduce -> scalar.
"""

from contextlib import ExitStack

import ml_dtypes
import numpy as np

import concourse.bass as bass
import concourse.tile as tile
from concourse import bacc, mybir
from concourse.bass_utils import run_bass_kernel_spmd

F32 = mybir.dt.float32
BF16 = mybir.dt.bfloat16
I32 = mybir.dt.int32
AF = mybir.ActivationFunctionType
OP = mybir.AluOpType
AX = mybir.AxisListType

N, C, D = 65536, 2048, 64
NCORES = 8
NS = N // NCORES  # 8192 rows per core
P = 128           # partitions
T = NS // P       # 64 row-tiles per core
FD = 512          # fp32 psum bank free dim
NB = C // FD      # 4 matmuls per row-tile

KA = D + 2        # 64 dims + e2 row + ones row
HC = C // 2

# tiles whose accumulation runs on ACT (Identity + accum_out) instead of
# the DVE stt, to balance the two engines
ACT_ASSIST = frozenset({21, 43})


def _body(tc, out, eT, dlabT, cT, onesrow):
    nc = tc.nc
    with ExitStack() as ctx:
        const = ctx.enter_context(tc.tile_pool(name="const", bufs=1))

        eTa = const.tile([KA, NS], BF16)    # [66, 8192] rows: e^T, e2, 1
        chat = const.tile([KA, C], BF16)    # [66, 2048] rows: -2c^T, 1, c2
        craw = const.tile([D, C], BF16)     # raw c^T (bf16)
        csqb = const.tile([D, C], BF16)     # 4c^2 for the ones-colsum matmul
        c2sb = const.tile([1, C], BF16)     # colsum (at partition 0)
        oquart = const.tile([P, 1], BF16)   # 0.25 (colsum un-scales the 4c^2)
        ones = const.tile([P, 1], F32)
        dall = const.tile([P, T], F32)
        macc = const.tile([P, T], F32)
        rowloss = const.tile([P, T], F32)
        rowtot = const.tile([P, 1], F32)
        outsb = const.tile([1, 1], F32)

        nc.sync.dma_start(dall[:], dlabT)
        nc.vector.memset(ones[:], 1.0)
        nc.vector.memset(oquart[:], 0.25)

        mm_ctx = tc.tile_pool(name="mm", bufs=2, space="PSUM")
        mm_pool = mm_ctx.__enter__()

        # chat build. The colsum psum row borrows an mm-pool slot so the
        # main-loop psum allocation isn't serialized behind a pool release.
        c2ps_full = mm_pool.tile([P, C], F32, name="ps", tag="ps")
        c2ps = c2ps_full[0:1, :]
        nc.sync.dma_start(craw[:], cT)
        nc.sync.dma_start(chat[D:D + 1, :], onesrow)
        # chat rows 0..63 = -2c (exact bf16 scaling); csqb = chat*chat =
        # 4c^2; the 0.25-ones colsum then yields c2 directly
        nc.vector.tensor_scalar_mul(chat[0:D, :], craw[:], -2.0)
        nc.vector.tensor_mul(csqb[:], chat[0:D, :], chat[0:D, :])
        for k in range(NB):
            sl = slice(k * FD, (k + 1) * FD)
            nc.tensor.matmul(
                c2ps[:, sl], lhsT=oquart[0:D, :], rhs=csqb[:, sl],
                start=True, stop=True,
            )
            nc.vector.tensor_copy(c2sb[:, sl], c2ps[:, sl])
            nc.sync.dma_start(chat[D + 1:KA, sl], c2sb[:, sl])

        # e loads: head chunk first so tile 0 starts early
        nc.sync.dma_start(eTa[:, 0:8 * P], eT[:, 0:8 * P])

        dist_pool = ctx.enter_context(tc.tile_pool(name="dist", bufs=7))
        z_pool = ctx.enter_context(tc.tile_pool(name="z", bufs=5))

        for t in range(T):
            if t == 4:
                nc.sync.dma_start(eTa[:, 8 * P:], eT[:, 8 * P:])
            ps = mm_pool.tile([P, C], F32, name="ps")
            lhsT = eTa[:, t * P:(t + 1) * P]
            for k in range(NB):
                nc.tensor.matmul(
                    ps[:, k * FD:(k + 1) * FD],
                    lhsT=lhsT,
                    rhs=chat[:, k * FD:(k + 1) * FD],
                    start=True, stop=True,
                )
            dist = dist_pool.tile([P, C], BF16, name="dist")
            nc.scalar.activation(dist[:], ps[:], AF.Sqrt)
            if t in ACT_ASSIST:
                # DVE min at 4x over the full tile; ACT sums
                z = z_pool.tile([P, C], BF16, name="z")
                nc.vector.tensor_scalar(
                    out=z[:], in0=dist[:],
                    scalar1=dall[:, t:t + 1], scalar2=None, op0=OP.min,
                )
                nc.scalar.activation(
                    z[:], z[:], AF.Identity,
                    accum_out=macc[:, t:t + 1],
                )
            else:
                # half-min at 4x, then stt: min(dist_R, d) + u, accum
                u = z_pool.tile([P, HC], BF16, name="u")
                nc.vector.tensor_scalar(
                    out=u[:], in0=dist[:, 0:HC],
                    scalar1=dall[:, t:t + 1], scalar2=None, op0=OP.min,
                )
                zz = z_pool.tile([P, HC], BF16, name="zz")
                nc.vector.scalar_tensor_tensor(
                    out=zz[:], in0=dist[:, HC:C],
                    scalar=dall[:, t:t + 1], in1=u[:],
                    op0=OP.min, op1=OP.add,
                    accum_out=macc[:, t:t + 1],
                )

        mm_ctx.__exit__(None, None, None)

        # rowloss = C*dall - macc ; loss_partial = sum_{p,t} rowloss
        nc.vector.scalar_tensor_tensor(
            out=rowloss[:], in0=dall[:], scalar=float(C), in1=macc[:],
            op0=OP.mult, op1=OP.subtract,
        )
        nc.vector.tensor_reduce(rowtot[:], rowloss[:], axis=AX.X, op=OP.add)
        with tc.tile_pool(name="fin", bufs=1, space="PSUM") as finp:
            fin = finp.tile([1, 1], F32)
            nc.tensor.matmul(fin[:], lhsT=rowtot[:], rhs=ones[:],
                             start=True, stop=True)
            nc.scalar.copy(outsb[:], fin[:])
        nc.sync.dma_start(out, outsb[:])


_NC_CACHE = {}


def build_nc():
    if "nc" in _NC_CACHE:
        return _NC_CACHE["nc"]
    nc = bacc.Bacc(
        "TRN2", target_bir_lowering=False, debug=False, enable_asserts=False
    )
    eT = nc.dram_tensor("eT", [KA, NS], BF16, kind="ExternalInput").ap()
    dlabT = nc.dram_tensor("dlabT", [P, T], F32, kind="ExternalInput").ap()
    cT = nc.dram_tensor("cT", [D, C], BF16, kind="ExternalInput").ap()
    onesrow = nc.dram_tensor("onesrow", [1, C], BF16, kind="ExternalInput").ap()
    out = nc.dram_tensor("out", [1, 1], F32, kind="ExternalOutput").ap()
    with tile.TileContext(nc) as tc:
        _body(tc, out, eT, dlabT, cT, onesrow)
    nc.compile()
    _NC_CACHE["nc"] = nc
    return nc


def make_in_maps(embeddings, centers, labels):
    e = np.ascontiguousarray(np.asarray(embeddings, dtype=np.float32))
    c = np.ascontiguousarray(np.asarray(centers, dtype=np.float32))
    lab = np.asarray(labels).astype(np.int64)
    assert e.shape == (N, D) and c.shape == (C, D) and lab.shape == (N,)
    cT = np.ascontiguousarray(c.T.astype(ml_dtypes.bfloat16))
    onesrow = np.ones((1, C), ml_dtypes.bfloat16)
    diff = e - c[lab]
    dlab = np.sqrt(np.maximum((diff * diff).sum(1), 1e-12)).astype(np.float32)
    e2 = (e * e).sum(1)
    in_maps = []
    for core in range(NCORES):
        sl = slice(core * NS, (core + 1) * NS)
        es = e[sl]
        eTa = np.ones((KA, NS), np.float32)
        eTa[0:D] = es.T
        eTa[D] = e2[sl]
        eTa = eTa.astype(ml_dtypes.bfloat16)
        in_maps.append({
            "eT": eTa,
            "dlabT": np.ascontiguousarray(
                dlab[sl].reshape(T, P).T.astype(np.float32)),
            "cT": cT,
            "onesrow": onesrow,
        })
    return in_maps


def run(embeddings, centers, labels, **kw):
    nc = build_nc()
    in_maps = make_in_maps(embeddings, centers, labels)
    res = run_bass_kernel_spmd(nc, in_maps, core_ids=list(range(NCORES)), **kw)
    total = float(sum(float(r["out"][0, 0]) for r in res.results))
    return np.float32(total), res


def kernel(embeddings, centers, labels):
    val, _ = run(embeddings, centers, labels)
    return val


# revision 3
# speedup vs baseline: 1.0094x; 1.0094x over previous
"""BallLoss Trainium2 kernel v6 (8-core data-parallel SPMD).

loss = sum_{i,j} relu(d_i - d_ij),  d_ij = ||e_i - c_j||, d_i = d_{i,label_i}

Per-core formulation (rows sharded along N across 8 cores, centers
replicated), using the identity
    sum_j relu(d_i - d_ij) = C*d_i - sum_j min(d_ij, d_i).

Per [128, 2048] row-tile:
  - PE:   full d2[i,j] = e2_i + c2_j - 2*e_i.c_j via one augmented bf16
          matmul: lhsT = [e; e2; 1]^T (stationary, K=66),
          rhs = [-2c; 1; c2]^T. Both norm terms ride the contraction,
          so no ACT bias is needed.
  - ACT:  dist = sqrt(psum), PSUM -> SBUF bf16, one op per tile
          (~1.85us). ACT and PE are the pacing engines.
  - DVE (most tiles): u = min(dist_left, d_i) at 4x over the left half,
          then scalar_tensor_tensor computes min(dist_right, d_i) + u
          with accum_out -> macc[:, t] = sum_j min(d_ij, d_i). The
          accum variant runs 1x but only over half the tile.
  - ACT-assist tiles (balance): DVE min at 4x over the full tile,
          ACT Identity + accum_out does the sum.

d_i arrives precomputed from the host (exact fp32 ||e_i - c_lab_i||,
one numpy gather+norm inside kernel()): the on-device alternatives all
lose — per-tile indirect-DMA gathers are throttled by DMA-ring credits
to ~2.2-2.8us/tile (slower than the tile pace, stalling the in-order
DVE/ACT queues through the dist pool), multi-offset batched indirect
DMA corrupts data on HW, and the dma_gather ext-isa wedges the device
in this runtime. No gpsimd compute at all (gpsimd elementwise stalls
2-port DVE ops on the shared SBUF port).

chat ([-2c; 1; c2] bf16) is host-prepared alongside the e-side
layout, so the kernel has no build phase at all: two DMAs and the
tile pipeline starts.

Final: rowloss = C*dall - macc (small stt), DVE free reduce + PE
ones-matmul partition reduce -> scalar.
"""

from contextlib import ExitStack

import ml_dtypes
import numpy as np

import concourse.bass as bass
import concourse.tile as tile
from concourse import bacc, mybir
from concourse.bass_utils import run_bass_kernel_spmd

F32 = mybir.dt.float32
BF16 = mybir.dt.bfloat16
I32 = mybir.dt.int32
AF = mybir.ActivationFunctionType
OP = mybir.AluOpType
AX = mybir.AxisListType

N, C, D = 65536, 2048, 64
NCORES = 8
NS = N // NCORES  # 8192 rows per core
P = 128           # partitions
T = NS // P       # 64 row-tiles per core
FD = 512          # fp32 psum bank free dim
NB = C // FD      # 4 matmuls per row-tile

KA = D + 2        # 64 dims + e2 row + ones row
HC = C // 2

# tiles whose accumulation runs on ACT (Identity + accum_out) instead of
# the DVE stt; empty — ACT paces the kernel, DVE fits underneath
ACT_ASSIST = frozenset()


def _body(tc, out, eT, dlabT, chatd):
    nc = tc.nc
    with ExitStack() as ctx:
        const = ctx.enter_context(tc.tile_pool(name="const", bufs=1))

        eTa = const.tile([KA, NS], BF16)    # [66, 8192] rows: e^T, e2, 1
        chat = const.tile([KA, C], BF16)    # [66, 2048] rows: -2c^T, 1, c2
        ones = const.tile([P, 1], F32)
        dall = const.tile([P, T], F32)
        macc = const.tile([P, T], F32)
        rowloss = const.tile([P, T], F32)
        rowtot = const.tile([P, 1], F32)
        outsb = const.tile([1, 1], F32)

        nc.sync.dma_start(chat[:], chatd)
        nc.sync.dma_start(dall[:], dlabT)
        nc.vector.memset(ones[:], 1.0)
        # e loads: head chunk first so tile 0 starts early
        nc.sync.dma_start(eTa[:, 0:8 * P], eT[:, 0:8 * P])

        mm_ctx = tc.tile_pool(name="mm", bufs=2, space="PSUM")
        mm_pool = mm_ctx.__enter__()

        dist_pool = ctx.enter_context(tc.tile_pool(name="dist", bufs=8))
        z_pool = ctx.enter_context(tc.tile_pool(name="z", bufs=5))

        for t in range(T):
            if t == 4:
                nc.sync.dma_start(eTa[:, 8 * P:], eT[:, 8 * P:])
            ps = mm_pool.tile([P, C], F32, name="ps")
            lhsT = eTa[:, t * P:(t + 1) * P]
            for k in range(NB):
                nc.tensor.matmul(
                    ps[:, k * FD:(k + 1) * FD],
                    lhsT=lhsT,
                    rhs=chat[:, k * FD:(k + 1) * FD],
                    start=True, stop=True,
                )
            dist = dist_pool.tile([P, C], BF16, name="dist")
            nc.scalar.activation(dist[:], ps[:], AF.Sqrt)
            if t in ACT_ASSIST:
                # DVE min at 4x over the full tile; ACT sums
                z = z_pool.tile([P, C], BF16, name="z")
                nc.vector.tensor_scalar(
                    out=z[:], in0=dist[:],
                    scalar1=dall[:, t:t + 1], scalar2=None, op0=OP.min,
                )
                nc.scalar.activation(
                    z[:], z[:], AF.Identity,
                    accum_out=macc[:, t:t + 1],
                )
            else:
                # half-min at 4x, then stt: min(dist_R, d) + u, accum
                u = z_pool.tile([P, HC], BF16, name="u")
                nc.vector.tensor_scalar(
                    out=u[:], in0=dist[:, 0:HC],
                    scalar1=dall[:, t:t + 1], scalar2=None, op0=OP.min,
                )
                zz = z_pool.tile([P, HC], BF16, name="zz")
                nc.vector.scalar_tensor_tensor(
                    out=zz[:], in0=dist[:, HC:C],
                    scalar=dall[:, t:t + 1], in1=u[:],
                    op0=OP.min, op1=OP.add,
                    accum_out=macc[:, t:t + 1],
                )

        mm_ctx.__exit__(None, None, None)

        # rowloss = C*dall - macc ; loss_partial = sum_{p,t} rowloss
        nc.vector.scalar_tensor_tensor(
            out=rowloss[:], in0=dall[:], scalar=float(C), in1=macc[:],
            op0=OP.mult, op1=OP.subtract,
        )
        nc.vector.tensor_reduce(rowtot[:], rowloss[:], axis=AX.X, op=OP.add)
        with tc.tile_pool(name="fin", bufs=1, space="PSUM") as finp:
            fin = finp.tile([1, 1], F32)
            nc.tensor.matmul(fin[:], lhsT=rowtot[:], rhs=ones[:],
                             start=True, stop=True)
            nc.scalar.copy(outsb[:], fin[:])
        nc.sync.dma_start(out, outsb[:])


_NC_CACHE = {}


def build_nc():
    if "nc" in _NC_CACHE:
        return _NC_CACHE["nc"]
    nc = bacc.Bacc(
        "TRN2", target_bir_lowering=False, debug=False, enable_asserts=False
    )
    eT = nc.dram_tensor("eT", [KA, NS], BF16, kind="ExternalInput").ap()
    dlabT = nc.dram_tensor("dlabT", [P, T], F32, kind="ExternalInput").ap()
    chatd = nc.dram_tensor("chatd", [KA, C], BF16, kind="ExternalInput").ap()
    out = nc.dram_tensor("out", [1, 1], F32, kind="ExternalOutput").ap()
    with tile.TileContext(nc) as tc:
        _body(tc, out, eT, dlabT, chatd)
    nc.compile()
    _NC_CACHE["nc"] = nc
    return nc


def make_in_maps(embeddings, centers, labels):
    e = np.ascontiguousarray(np.asarray(embeddings, dtype=np.float32))
    c = np.ascontiguousarray(np.asarray(centers, dtype=np.float32))
    lab = np.asarray(labels).astype(np.int64)
    assert e.shape == (N, D) and c.shape == (C, D) and lab.shape == (N,)
    cb = c.astype(ml_dtypes.bfloat16).astype(np.float32)
    chat = np.empty((KA, C), np.float32)
    chat[0:D] = -2.0 * cb.T
    chat[D] = 1.0
    chat[D + 1] = (cb * cb).sum(1)
    chat = np.ascontiguousarray(chat.astype(ml_dtypes.bfloat16))
    diff = e - c[lab]
    dlab = np.sqrt(np.maximum((diff * diff).sum(1), 1e-12)).astype(np.float32)
    e2 = (e * e).sum(1)
    in_maps = []
    for core in range(NCORES):
        sl = slice(core * NS, (core + 1) * NS)
        es = e[sl]
        eTa = np.ones((KA, NS), np.float32)
        eTa[0:D] = es.T
        eTa[D] = e2[sl]
        eTa = eTa.astype(ml_dtypes.bfloat16)
        in_maps.append({
            "eT": eTa,
            "dlabT": np.ascontiguousarray(
                dlab[sl].reshape(T, P).T.astype(np.float32)),
            "chatd": chat,
        })
    return in_maps


def run(embeddings, centers, labels, **kw):
    nc = build_nc()
    in_maps = make_in_maps(embeddings, centers, labels)
    res = run_bass_kernel_spmd(nc, in_maps, core_ids=list(range(NCORES)), **kw)
    total = float(sum(float(r["out"][0, 0]) for r in res.results))
    return np.float32(total), res


def kernel(embeddings, centers, labels):
    val, _ = run(embeddings, centers, labels)
    return val
